# revision 55
# baseline (speedup 1.0000x reference)
"""KANConv2d Trainium2 kernel (8-core data-parallel over batch).

Math: on the uniform efficient-kan grid (u = x/0.4 + 5.5, knots u=0..11)
every B-spline basis is a shift of ONE cardinal cubic B-spline:
B_g(x) = b3(u - g), and with s = |u - g - 2|

    6*b3 = (2-s)+^3 - 4*(1-s)+^3   (exactly zero for s >= 2)

So instead of the 14 truncated-power feature maps (whose weight-folded
telescopes force 69 matmul jobs), evaluate the 8 bases DIRECTLY as 4
pair-tiles (2 bases per 128-partition tile via per-partition Abs bias)
plus the silu tile: 9 feature maps -> 42 matmul jobs, cutting the PE
floor from ~88us to ~55us. /6 is folded into the host weights.

Everything downstream of x runs in fp16 (10-bit mantissa; DVE gets its
2x/4x packed modes, PE runs 1 cycle/row, and end-to-end error is
~8e-4): per pair s=Abs(2.5x+b) [ACT, per-partition bias], A=Relu(2-s)
and Bt=c4*(1-s)+ [ACT, c4=4^(1/3) so Bt^3=4B^3], squares/cubes
[DVE tensor_mul 2x], combine ft=A3-Bt3 [DVE tensor_sub 2x]. Pair-1
section A instead runs ENTIRELY on DVE straight from x (negated D/E
form: u'=affine ts, s=max(u',-u') via a second negated-bias affine +
tensor_max since abs_max/pow are invalid DVE ISA ops; 4E3-D3 ==
A3-4B3) so its chain starts at x-arrival, ~4us before the ACT queue
frees up; its section B uses DVE tensor_scalar B=(A-1)+ +
scalar_tensor_tensor combine. PSUM accumulates fp32.

Host-side (layout only, no compute): x is padded to the 58-wide map,
duplicated into both partition halves, and cast to fp16, so the x DMAs
are 4 coalesced full-partition transfers with no border memsets and no
small-element (<512B) descriptor penalty. Weights are folded
(spline_weight*scaler/6) and packed per matmul job in fp16.

Matmul rhs uses a strided [rows x 56] view so no garbage pad columns
are streamed. PSUM banks are uneven (6x9 rows + 2 rows) so the
end-of-kernel drain after the very last matmul moves only 112 columns;
drain copies/DMAs alternate DVE/SP and ACT queues to avoid dispatch
serialization. PE warm-up matmuls keep the busy-streak alive so real
matmuls price at the full p-state (verified: all 252 big matmuls at
210ns, 42 small at 47ns = the 54.9us floor). The silu tile covers
banks 0-2 from section-A columns while section B is still computing.
"""
import numpy as np

import concourse.bass as bass
import concourse.mybir as mybir
from concourse.tile import TileContext

# ---- problem constants (hardcoded per harness contract) ----
B, C, H, W = 8, 64, 56, 56
OC = 128
GRID_SIZE, SPLINE_ORDER = 5, 3
GRID_LO, GRID_HI = -1.0, 1.0
HSTEP = (GRID_HI - GRID_LO) / GRID_SIZE        # 0.4
T0 = GRID_LO - SPLINE_ORDER * HSTEP            # -2.2
WP = W + 2                                     # 58 padded width
PADFLAT = WP * WP + 4                          # 3368 padded/rounded
# psum bank row counts: 9-row banks (504 <= 512 psum fp32) and a tiny
# final bank so the end-of-kernel drain (copy + DMA after the last
# matmul) moves 4x less data
BROWS = [9, 9, 9, 9, 9, 9, 2]
BROW0 = [sum(BROWS[:k]) for k in range(len(BROWS))]
NCH = len(BROWS)                               # 7 banks: 6*9+2 = 56 rows
NT = 5                                         # feature tiles: silu + 4 pairs
F32 = mybir.dt.float32
F16 = mybir.dt.float16
M_DT = F16                                     # matmul/feature dtype
ALU = mybir.AluOpType

NWARM_TINY = 30                                # burn the 36-deep price burst
NWARM = 50                                     # PE warm-up matmuls (medium)
PAD_T0HB = 0                                   # pads before t0h banks 3-6
PAD_T0FA = 0                                   # pads before t0f banks 0-2
PAD_T0FB = 0                                   # pads before t0f banks 3-6
PAD_P1 = 0                                     # pads before pair-1 group
PAD_P2 = 0
PAD_TAIL = 0                                    # pads between pair groups
CA = 1682                                      # column split (banks 0-2 | 3-6)


def _jobs():
    """Matmul job list: (tile, rows, off, blocks). blocks[i] covers lhsT
    rows 64i:64i+64; ("base", kh, kw) or ("bas", g, kh, kw)."""
    jobs = []
    # t0 half jobs first (need only the unshifted silu half)
    for kh in range(3):
        jobs.append((0, 64, kh * WP + 2, [("base", kh, 2)]))
    # t0 full jobs: rows 0:64 tap kw=0, rows 64:128 (shifted copy) tap kw=1
    for kh in range(3):
        jobs.append((0, 128, kh * WP, [("base", kh, 0), ("base", kh, 1)]))
    for i in range(1, NT):
        glo = 2 * (i - 1)
        for s in range(9):
            kh, kw = s // 3, s % 3
            jobs.append((i, 128, kh * WP + kw,
                         [("bas", glo, kh, kw), ("bas", glo + 1, kh, kw)]))
    return jobs


_JOBS = _jobs()
NJ = len(_JOBS)                                # 42
_TILE_J0 = [min(j for j, job in enumerate(_JOBS) if job[0] == t) for t in range(NT)]
_TILE_J1 = [max(j for j, job in enumerate(_JOBS) if job[0] == t) + 1 for t in range(NT)]


def _patch_tile_drain():
    """walrus in this container rejects sem waits on InstDrain (CTRL_NO
    struct): move the end-of-kernel drain waits onto single-wait NOPs."""
    import bass_rust

    def _drain_and_barrier(self, tick_clock, wait_clock):
        collector = self.nc.sync.nop(nofuse=True, hint="drain_waits")
        wait_clock.add_sem_waits(
            collector.ins, bass_rust.ScopedClock({None: tick_clock.global_clock})
        )
        waits = list(collector.ins.sync_info.on_wait)
        collector.ins.sync_info = mybir.SyncInfo(on_wait=waits[:1], on_update=[])
        # spill the remaining waits two-at-a-time onto EventSemaphore
        # instructions (walrus cap: 2 waits there vs 1 on a NOP), spread
        # round-robin over all four sequencers: the kernel ends when every
        # queue drains, so the post-last-semaphore dispatch chain quarters
        rest = waits[1:]
        engs = [mybir.EngineType.SP, mybir.EngineType.Activation,
                mybir.EngineType.DVE, mybir.EngineType.Pool]
        for i in range(0, len(rest), 2):
            ev = bass_rust.InstEventSemaphore(
                name=f"drainev-{i}", engine=engs[(i // 2) % 4],
                ins=[], outs=[],
                sync_info=mybir.SyncInfo(on_wait=rest[i:i + 2], on_update=[]),
            )
            self.nc.add_instruction(ev)
        self.nc.sync.drain()
        popped = self.nc._tile_sem_poison_stack.pop()
        assert popped is self._sem_poison

    TileContext._drain_and_barrier = _drain_and_barrier


_patch_tile_drain()


def _split_excess_waits(nc):
    """This walrus caps sync waits at 1/instruction (2 for EventSemaphore).
    Spill excess waits onto EventSemaphore insts inserted just before the
    overloaded instruction on the same engine."""
    import bass_rust

    counter = [0]
    for func in nc.m.functions:
        for bb in func.blocks:
            insts = bb.instructions
            out = []
            changed = False
            for inst in insts:
                si = getattr(inst, "sync_info", None)
                waits = list(si.on_wait) if si is not None else []
                cap = 2 if isinstance(inst, bass_rust.InstEventSemaphore) else 1
                if len(waits) > cap:
                    excess = waits[cap:]
                    for i in range(0, len(excess), 2):
                        counter[0] += 1
                        ev = bass_rust.InstEventSemaphore(
                            name=f"evspill-{counter[0]}",
                            engine=inst.engine,
                            ins=[], outs=[],
                            sync_info=mybir.SyncInfo(
                                on_wait=excess[i:i + 2], on_update=[]),
                        )
                        out.append(ev)
                    inst.sync_info = mybir.SyncInfo(
                        on_wait=waits[:cap], on_update=list(si.on_update))
                    changed = True
                out.append(inst)
            if changed:
                bb.instructions = out


def _host_weights(base_weight, spline_weight, spline_scaler):
    """Fold spline scaler and the /6 into per-job lhsT blocks Wt[row, j, o]."""
    scaled = (spline_weight.astype(np.float64)
              * spline_scaler.astype(np.float64)[..., None] / 6.0)
    W24 = scaled.reshape(OC, C, 3, 3, 8)                        # (O,c,kh,kw,g)
    bw4 = base_weight.astype(np.float64).reshape(OC, C, 3, 3)
    wt = np.zeros((128, NJ, OC), dtype=np.float64)
    for j, (t, rows, off, blocks) in enumerate(_JOBS):
        for bi, blk in enumerate(blocks):
            if blk[0] == "base":
                _, kh, kw = blk
                blockw = bw4[:, :, kh, kw]                      # (O, C)
            else:
                _, g, kh, kw = blk
                blockw = W24[:, :, kh, kw, g]                   # (O, C)
            wt[64 * bi:64 * bi + 64, j, :] = blockw.T
    return wt.astype(np.float16)


def _build_nc():
    nc = bass.Bass()
    x_in = nc.declare_dram_parameter("x", [128, PADFLAT], F16, isOutput=False)
    wt_in = nc.declare_dram_parameter("wt", [128, NJ, OC], M_DT, isOutput=False)
    out = nc.declare_dram_parameter("out", [OC, H, W], F32, isOutput=True)

    AF = mybir.ActivationFunctionType
    with TileContext(nc) as tc:
        with (
            tc.tile_pool(name="w", bufs=1) as wpool,
            tc.tile_pool(name="xf", bufs=1) as xfpool,
            tc.tile_pool(name="sp", bufs=2) as spool,
            tc.tile_pool(name="ap", bufs=2) as apool,
            tc.tile_pool(name="a2p", bufs=2) as a2pool,
            tc.tile_pool(name="bp", bufs=2) as bpool,
            tc.tile_pool(name="ob", bufs=4) as opool,
            tc.tile_pool(name="psum", bufs=1, space="PSUM") as psumpool,
        ):
            # bias vector built by on-chip memsets (no DMA slot):
            # cols 0..3 = pair Abs biases (3.5 - g per partition half),
            # col 4 = 2.0 (the A = relu(2 - s) bias)
            # PE warm-up: keep the PE busy-streak alive from ~2us so real
            # matmuls price at the full p-state (warm memset first on the
            # Pool queue so warms start immediately).
            warm = wpool.tile([64, 128], M_DT, tag="warm")
            nc.gpsimd.memset(warm[:], 0.0)
            psum_w = psumpool.tile([64, 64], F32, tag="pwarm")

            bias_sb = wpool.tile([128, NT + 1], F32, tag="bias_sb")
            for i in range(1, NT):
                glo = 2 * (i - 1)
                nc.gpsimd.memset(bias_sb[0:64, i - 1:i], 3.5 - glo)
                nc.gpsimd.memset(bias_sb[64:128, i - 1:i], 3.5 - (glo + 1))
            nc.gpsimd.memset(bias_sb[:, NT - 1:NT], 2.0)
            C4 = 4.0 ** (1.0 / 3.0)
            nc.gpsimd.memset(bias_sb[:, NT:NT + 1], C4)
            nbias = wpool.tile([128, 1], F32, tag="nbias")
            nc.gpsimd.memset(nbias[0:64, :], -3.5)
            nc.gpsimd.memset(nbias[64:128, :], -2.5)
            # x arrives pre-padded and pre-duplicated (host layout
            # transform): [128, PADFLAT] fp16, halves identical
            xpad = xfpool.tile([128, PADFLAT], F16, tag="xpad")

            def warms(n, ap=64):
                for _ in range(n):
                    nc.tensor.matmul(psum_w[0:64, 0:ap], warm[0:64, 0:64],
                                     warm[0:64, 64:64 + ap],
                                     start=True, stop=True)

            warms(NWARM_TINY, ap=8)
            warms(NWARM)

            w_sb = [wpool.tile([128, (_TILE_J1[t] - _TILE_J0[t]) * OC], M_DT,
                               tag=f"w{t}", name=f"w{t}") for t in range(NT)]

            # Single SP DMA queue, ordered by need: x first-half section A,
            # t0 weights, dup half A, then B halves, then pair weights.
            CH2 = CA // 2
            nc.sync.dma_start(xpad[:, 0:CH2], x_in[:, 0:CH2])
            nc.sync.dma_start(xpad[:, CH2:CA], x_in[:, CH2:CA])
            nc.sync.dma_start(xpad[:, CA:2525], x_in[:, CA:2525])
            nc.sync.dma_start(xpad[:, 2525:PADFLAT], x_in[:, 2525:PADFLAT])
            nc.sync.dma_start(w_sb[0][0:64, 0:3 * OC],
                              wt_in[0:64, 0:3, :].rearrange("p j o -> p (j o)"))
            nc.sync.dma_start(w_sb[0][:, 3 * OC:6 * OC],
                              wt_in[:, 3:6, :].rearrange("p j o -> p (j o)"))

            ft = [xfpool.tile([128, PADFLAT], M_DT, tag=f"ft{t}", name=f"ft{t}")
                  for t in range(NT)]
            # silu tile: rows 0:64 plain; rows 64:128 get the 1-col-shifted
            # copy via SB->SB DMAs (DMA engines have slack; ACT does not)
            nc.scalar.activation(ft[0][0:64, 0:CA // 2], xpad[0:64, 0:CA // 2], AF.Silu)
            nc.scalar.activation(ft[0][0:64, CA // 2:CA], xpad[0:64, CA // 2:CA], AF.Silu)
            with tc.high_priority():
                nc.sync.dma_start(ft[0][64:128, 0:CA - 1], ft[0][0:64, 1:CA])

            # ---- basis pair tiles ----
            # per (pair, section): s=Abs(2.5x+b) [ACT], A=relu(2-s) [ACT],
            # B=(A-1)+ [DVE ts], squares on ACT (pairs 1,2) or DVE (3,4),
            # cubes+combine on DVE (bf16 2x).
            def pair_act(i, c0, c1, with_bt=True):
                w = c1 - c0
                s_t = spool.tile([128, CA + 8], F16, tag="s", name=f"s{i}")
                a_t = apool.tile([128, CA + 8], F16, tag="a", name=f"a{i}")
                nc.scalar.activation(s_t[:, 0:w], xpad[:, c0:c1], AF.Abs,
                                     scale=1.0 / HSTEP, bias=bias_sb[:, i - 1:i])
                nc.scalar.activation(a_t[:, 0:w], s_t[:, 0:w], AF.Relu,
                                     scale=-1.0, bias=bias_sb[:, NT - 1:NT])
                if not with_bt:
                    return a_t, None
                # Bt = c4*(1-s)+ so Bt^3 = 4*(1-s)+^3 and the combine is a
                # plain 2x-mode tensor_sub
                b_t = bpool.tile([128, CA + 8], F16, tag="b", name=f"b{i}")
                nc.scalar.activation(b_t[:, 0:w], s_t[:, 0:w], AF.Relu,
                                     scale=-(4.0 ** (1.0 / 3.0)),
                                     bias=bias_sb[:, NT:NT + 1])
                return a_t, b_t

            def pair_dve(i, c0, c1, ab):
                a_t, b_t = ab
                w = c1 - c0
                a2_t = a2pool.tile([128, CA + 8], F16, tag="a2", name=f"a2{i}")
                b2_t = bpool.tile([128, CA + 8], F16, tag="b2", name=f"b2{i}")
                if b_t is None:
                    # startup path: keep ACT off the critical chain; B via
                    # DVE tensor_scalar, combine via scalar_tensor_tensor
                    b_t = bpool.tile([128, CA + 8], F16, tag="b", name=f"b{i}")
                    nc.vector.tensor_scalar(b_t[:, 0:w], a_t[:, 0:w], 1.0, 0.0,
                                            ALU.subtract, ALU.max)
                    nc.vector.tensor_mul(a2_t[:, 0:w], a_t[:, 0:w], a_t[:, 0:w])
                    nc.vector.tensor_mul(b2_t[:, 0:w], b_t[:, 0:w], b_t[:, 0:w])
                    nc.vector.tensor_mul(a_t[:, 0:w], a2_t[:, 0:w], a_t[:, 0:w])
                    nc.vector.tensor_mul(b_t[:, 0:w], b2_t[:, 0:w], b_t[:, 0:w])
                    nc.vector.scalar_tensor_tensor(ft[i][:, c0:c1], b_t[:, 0:w],
                                                   -4.0, a_t[:, 0:w],
                                                   ALU.mult, ALU.add)
                    return
                nc.vector.tensor_mul(a2_t[:, 0:w], a_t[:, 0:w], a_t[:, 0:w])
                nc.vector.tensor_mul(b2_t[:, 0:w], b_t[:, 0:w], b_t[:, 0:w])
                # cubes in place, then ft = A3 - Bt3
                nc.vector.tensor_mul(a_t[:, 0:w], a2_t[:, 0:w], a_t[:, 0:w])
                nc.vector.tensor_mul(b_t[:, 0:w], b2_t[:, 0:w], b_t[:, 0:w])
                nc.vector.tensor_sub(ft[i][:, c0:c1], a_t[:, 0:w], b_t[:, 0:w])


            def pair_de_dve(i, c0, c1):
                """Pure-DVE pipeline (negated D/E form): reads x directly so
                it starts before any ACT pass. 4E3-D3 == A3-4B3."""
                w = c1 - c0
                u_t = spool.tile([128, CA + 8], F16, tag="s", name=f"u{i}")
                d_t = apool.tile([128, CA + 8], F16, tag="a", name=f"d{i}")
                e_t = bpool.tile([128, CA + 8], F16, tag="b", name=f"e{i}")
                d2_t = a2pool.tile([128, CA + 8], F16, tag="a2", name=f"d2{i}")
                e2_t = bpool.tile([128, CA + 8], F16, tag="b2", name=f"e2{i}")
                v_t = bpool.tile([128, CA + 8], F16, tag="b2", name=f"v{i}")
                nc.vector.tensor_scalar(u_t[:, 0:w], xpad[:, c0:c1], 1.0 / HSTEP,
                                        bias_sb[:, i - 1:i], ALU.mult, ALU.add)
                nc.vector.tensor_scalar(v_t[:, 0:w], xpad[:, c0:c1], -1.0 / HSTEP,
                                        nbias[:, 0:1], ALU.mult, ALU.add)
                nc.vector.tensor_max(u_t[:, 0:w], u_t[:, 0:w], v_t[:, 0:w])
                nc.vector.tensor_scalar(d_t[:, 0:w], u_t[:, 0:w], 2.0, 0.0,
                                        ALU.subtract, ALU.min)
                nc.vector.tensor_scalar(e_t[:, 0:w], u_t[:, 0:w], 1.0, 0.0,
                                        ALU.subtract, ALU.min)
                nc.vector.tensor_mul(d2_t[:, 0:w], d_t[:, 0:w], d_t[:, 0:w])
                nc.vector.tensor_mul(e2_t[:, 0:w], e_t[:, 0:w], e_t[:, 0:w])
                nc.vector.tensor_mul(d_t[:, 0:w], d2_t[:, 0:w], d_t[:, 0:w])
                nc.vector.tensor_mul(e_t[:, 0:w], e2_t[:, 0:w], e_t[:, 0:w])
                nc.vector.scalar_tensor_tensor(ft[i][:, c0:c1], e_t[:, 0:w],
                                               4.0, d_t[:, 0:w],
                                               ALU.mult, ALU.subtract)

            # pair-1 section A runs entirely on DVE straight from x (no ACT
            # dependency: DVE starts at x-arrival, ~4us before ACT frees up);
            # section B in quarter subsections through ACT as before
            pair_de_dve(1, 0, CA)
            nc.scalar.activation(ft[0][0:64, CA:PADFLAT],
                                 xpad[0:64, CA:PADFLAT], AF.Silu)
            with tc.high_priority():
                nc.sync.dma_start(ft[0][64:128, CA - 1:PADFLAT - 1],
                                  ft[0][0:64, CA:PADFLAT])
            with tc.tile_wait_until(0.0080):
                for t in range(1, NT):
                    nc.sync.dma_start(
                        w_sb[t][:],
                        wt_in[:, _TILE_J0[t]:_TILE_J1[t], :]
                        .rearrange("p j o -> p (j o)"))
            a = pair_act(1, CA, 2525, with_bt=False)
            pair_dve(1, CA, 2525, a)
            a = pair_act(1, 2525, PADFLAT, with_bt=False)
            pair_dve(1, 2525, PADFLAT, a)
            for i in range(2, NT):
                aA = pair_act(i, 0, CA)
                pair_dve(i, 0, CA, aA)
                aB = pair_act(i, CA, PADFLAT)
                pair_dve(i, CA, PADFLAT, aB)

            psum = [psumpool.tile([128, BROWS[k] * W], F32, tag=f"pb{k}",
                                  name=f"pb{k}") for k in range(NCH)]

            def rhs_ap(t, rows, off, k):
                base = off + BROW0[k] * WP
                return (ft[t][0:rows, base:base + BROWS[k] * WP]
                        .rearrange("p (r c) -> p r c", c=WP)[:, :, 0:W])

            def emit_group(jobs, j0, first, last, banks=range(NCH)):
                for k in banks:
                    for jj, (t, rows, off, blocks) in enumerate(jobs):
                        lhsT = w_sb[t][0:rows, (j0 + jj - _TILE_J0[t]) * OC:
                                       (j0 + jj - _TILE_J0[t] + 1) * OC]
                        nc.tensor.matmul(psum[k][:], lhsT,
                                         rhs_ap(t, rows, off, k),
                                         start=(first and jj == 0),
                                         stop=(last and jj == len(jobs) - 1))
                    if last:
                        ob = opool.tile([128, BROWS[0] * W], F32, tag="ob",
                                        name=f"ob{k}")
                        nb = BROWS[k] * W
                        if k % 2 == 1:
                            nc.vector.tensor_copy(ob[:, 0:nb], psum[k][:])
                            nc.sync.dma_start(
                                out[:, BROW0[k]:BROW0[k] + BROWS[k], :]
                                .rearrange("o r c -> o (r c)"),
                                ob[:, 0:nb])
                        else:
                            nc.scalar.activation(ob[:, 0:nb], psum[k][:], AF.Copy)
                            nc.scalar.dma_start(
                                out[:, BROW0[k]:BROW0[k] + BROWS[k], :]
                                .rearrange("o r c -> o (r c)"),
                                ob[:, 0:nb])

            # t0 half jobs (plain silu) by bank phase, then t0 full jobs
            # (need shifted silu), then each pair across all 7 banks;
            # drains ride the last (pair 4) group
            bA, bB = range(0, 3), range(3, NCH)
            emit_group(_JOBS[0:3], 0, True, False, banks=bA)
            warms(PAD_T0HB)
            emit_group(_JOBS[0:3], 0, True, False, banks=[3])
            emit_group(_JOBS[0:3], 0, True, False, banks=range(4, NCH))
            warms(PAD_T0FA)
            emit_group(_JOBS[3:6], 3, False, False, banks=bA)
            warms(PAD_T0FB)
            emit_group(_JOBS[3:6], 3, False, False, banks=bB)
            warms(PAD_P1)
            for i in range(1, NT):
                if i == 2:
                    warms(PAD_P2)
                if i == 1:
                    for bk in (0, 1, 2):
                        emit_group(_JOBS[_TILE_J0[1]:_TILE_J1[1]], _TILE_J0[1],
                                   False, False, banks=[bk])
                    emit_group(_JOBS[_TILE_J0[1]:_TILE_J1[1]], _TILE_J0[1],
                               False, False, banks=range(3, NCH))
                    continue
                if i == NT - 1:
                    # spread the last two banks' stops so bank 5's drain
                    # clears the shared HWDGE/DMA path before bank 6 stops
                    emit_group(_JOBS[_TILE_J0[i]:_TILE_J1[i]], _TILE_J0[i],
                               False, True, banks=range(0, 6))
                    warms(PAD_TAIL)
                    emit_group(_JOBS[_TILE_J0[i]:_TILE_J1[i]], _TILE_J0[i],
                               False, True, banks=[6])
                    continue
                emit_group(_JOBS[_TILE_J0[i]:_TILE_J1[i]], _TILE_J0[i],
                           False, i == NT - 1)
    _split_excess_waits(nc)
    return nc


_CACHE = {}


def kernel(x, base_weight, spline_weight, spline_scaler):
    from concourse.bass_utils import run_bass_kernel_spmd

    x = np.asarray(x, dtype=np.float16)
    xp = np.zeros((B, 128, PADFLAT), dtype=np.float16)
    xv = xp[:, 0:C, :WP * WP].reshape(B, C, WP, WP)
    xv[:, :, 1:H + 1, 1:W + 1] = x
    xp[:, C:2 * C] = xp[:, 0:C]
    wt = _host_weights(
        np.asarray(base_weight, np.float32),
        np.asarray(spline_weight, np.float32),
        np.asarray(spline_scaler, np.float32),
    )
    if "nc" not in _CACHE:
        _CACHE["nc"] = _build_nc()
    nc = _CACHE["nc"]
    in_maps = [{"x": xp[b], "wt": wt} for b in range(B)]
    res = run_bass_kernel_spmd(nc, in_maps, list(range(B)))
    out = np.stack([res.results[b]["out"] for b in range(B)], axis=0)
    return out
